# revision 1
# baseline (speedup 1.0000x reference)
"""Trainium2 Bass kernel for nn_BertClassifier_37907381354985 (v4).

Span-pair classifier: for every valid span (i, j) with i <= j < i + 30 over
L=128 tokens, compute log_softmax(relu(x_i W1a + x_j W1b + ind*w1c + b1) W2 + b2).

Strategy (data-parallel over batch, 2 batches per core on 8 cores):
  * Algebraic restructuring: precompute AT = W1a^T X^T and CT = W1b^T X^T
    ([H1, L] per batch) on the tensor engine; every span's hidden vector is
    AT[:, i] + CT[:, j] -- spans grouped by width w = j - i make this a
    *shifted add* along the free axis (no gather).
  * The pred-span indicator decomposes as
        ind = 1{i>=s} - 1{j>e} + 1{i<s & j>e} + 1{i==s & j==e}
    The first two terms are rank-1 and fold into the AT/CT matmuls via
    augmented contraction rows (u[i]=1{i>=s}, ones, v[j]=1{j>e} appended to
    X^T; w1c/b1/-w1c rows appended to the weights).
  * Per-batch token ROTATION by e: device column i' holds token (i'+e)%L.
    Under this rotation the remaining sparse correction q (contained +
    exact spans) provably lands in STATIC windows -- column 0 plus a
    [(127,29),(1,29)] strided pattern (842 slots) -- instead of a 98x30
    region (2940 slots), cutting the q pass ~3.5x.  The rotation is free:
    the host permutes X^T columns and the output permutation.
  * The b2 ones-row of h is produced by the matmul itself: W1a/W1b get an
    extra H1 column (index 770) hitting only the augmented all-ones X^T
    row with weight 0.5 + 0.5, so AT[770,i']+CT[770,j'] == 1.0.
  * Constant loads are batched into ~10 wide DMAs (was 87): each DMA costs
    ~600ns serialized HWDGE descriptor-generation + SP sequencer time,
    which dominated the first 40us of the v3 timeline.  Small tensors
    (q rows, W2) are issued before the bulk weight chunks so the
    assembly/q pipeline is not gated on the full 2.8MB weight load.
  * PE pstate warmup: dummy matmuls during the DMA window keep the PE
    clock ramp going so phase-1 matmuls run at 2.4GHz, not 0.65-1.2GHz.
  * Engine split (tuned on the instruction cost model): even+some odd
    diagonal assembly, q-mul/add, 6 relu diagonals and softmax
    reduce/subtract on DVE (2x/4x modes); 24 relu diagonals, PSUM->SBUF
    copies, exp/ln on ACT; 6 odd diagonals on Pool.  The last two h tiles
    split their q/relu ops in halves so phase-3 matmuls drain early.
  * log_softmax over the free axis in fp32; max-subtraction skipped --
    logits are O(10) so exp cannot overflow.
TimelineSim: 63.3us/exec vs 119us for v3 (HW-validated ~121us/iter).
"""

import numpy as np

L = 128
D = 768
H1 = 770
H1E = H1 + 1              # augmented h width (col 770 == constant-one row)
OUT = 40
WMAX = 30
B = 16
NCORES = 8
BL = B // NCORES          # batches per core
HT = 110                  # h rows per k-tile
NK = 7                    # h k-tiles (6*110 + 111 = 771)
ND = 7                    # contraction tiles (6 * 128 data + 1 aug tile)
DAUG = ND * 128           # padded contraction rows
FDH = WMAX * L            # diagonal-major span slots per batch (3840)
NCH = FDH // L            # span chunks of 128 (= WMAX)
QC = 1 + 29 * 29          # compact q slots (rotated layout: col0 + 29x29 windows)
NW2 = WMAX // 2
CEXT = L + 30             # extended (wrapped) CT columns per batch (even: keeps 2x alignment)

_prog_cache = {}


def _f32(x):
    return np.ascontiguousarray(np.asarray(x, dtype=np.float32))


def _bf16(x):
    import ml_dtypes
    return np.ascontiguousarray(np.asarray(x, dtype=np.float32).astype(ml_dtypes.bfloat16))


def _view(base, col_off, dims):
    """Free-axis re-view of a 2D [P, F] SBUF access pattern.

    dims: list of (step, count) free dims, outer->inner.  Partition dim kept.
    """
    from concourse.ap import AP
    ap0 = list(base.ap)
    part = [list(ap0[0])]
    return AP(
        tensor=base.tensor,
        offset=base.offset + col_off,
        ap=part + [[int(s), int(c)] for s, c in dims],
    )


def _dram_view(t, off, dims):
    """Custom-stride DRAM access pattern (dim0 maps to SBUF partitions)."""
    from concourse.ap import AP
    base = t.ap()
    return AP(
        tensor=base.tensor,
        offset=base.offset + off,
        ap=[[int(s), int(c)] for s, c in dims],
    )


def _make_tc_class():
    import concourse.mybir as mybir
    from concourse.tile import TileContext
    from concourse.vector_clock import ScopedClock

    # --- TileContext variant for this container's walrus build, which encodes
    # at most ONE sync-wait condition per instruction.  Tile freely attaches
    # several waits to one instruction, so (a) every scheduled instruction
    # with more than one wait gets the excess hoisted onto same-engine NOPs
    # inserted directly before it, and (b) the kernel-tail drain (one wait per
    # logical processor) is split the same way.  Waits are AND conditions, so
    # any same-engine placement before the original instruction preserves the
    # happens-before edges.
    class SplitDrainTileContext(TileContext):
        def _split_multi_waits(self, ordered):
            for bb_name, insts in ordered.items():
                out_list = []
                for inst in insts:
                    si = getattr(inst, "sync_info", None)
                    waits = list(si.on_wait) if si is not None and si.on_wait else []
                    if len(waits) > 1:
                        for w in waits[:-1]:
                            nop = mybir.InstNoOp(
                                name=self.nc.get_next_instruction_name(),
                                engine=inst.engine,
                                sync_info=mybir.SyncInfo(on_wait=[w], on_update=[]),
                                text_hint="waitsplit",
                                bass_nofuse=True,
                            )
                            self.nc.register_instruction(nop, overwrite=True)
                            out_list.append(nop)
                        inst.sync_info = mybir.SyncInfo(
                            on_wait=[waits[-1]],
                            on_update=list(si.on_update or []),
                        )
                    out_list.append(inst)
                insts[:] = out_list

        def _lower_ordered_insts(self, ordered):
            self._split_multi_waits(ordered)
            super()._lower_ordered_insts(ordered)

        def _drain_and_barrier(self, tick_clock, wait_clock):
            drain_inst = self.nc.sync.drain()
            wait_clock.add_sem_waits(
                drain_inst.ins, ScopedClock({None: tick_clock.global_clock})
            )
            si = drain_inst.ins.sync_info
            waits = list(si.on_wait) if si is not None and si.on_wait else []
            if len(waits) > 1:
                drain_inst.ins.sync_info = mybir.SyncInfo(
                    on_wait=waits[:1], on_update=list(si.on_update or [])
                )
                for i in range(1, len(waits)):
                    nop = self.nc.sync.nop(nofuse=True, hint="drain_split")
                    nop.ins.sync_info = mybir.SyncInfo(
                        on_wait=waits[i : i + 1], on_update=[]
                    )
            self.nc.all_engine_barrier()
            assert self.sems is not None
            popped = self.nc._tile_sem_poison_stack.pop()
            assert popped is self._sem_poison
            self.nc.clear_and_free_semaphores(list(self.sems.allocated().values()))
            self.nc.all_engine_barrier()

    return SplitDrainTileContext


def _default_cfg():
    # engine split tuned against the TimelineSim cost model (63.3us/exec,
    # vs 119us for the v3 kernel) and validated for correctness on HW
    return {
        "p_odd_pool": 6,       # odd diagonals w=1..11 assembled on Pool
        "relu_act_diag": 8,    # relu: diagonals [0,8) on DVE, [8,30) on ACT
        "relu_pool_diag": 0,
        "qadd": "dve",
        "tile_order": "bk",
        "emission": "tiles",
        "loop_reps": 0,
        "wsplit": 3,           # wa/wc loads split into this many column chunks
        "cb_order": "a_first",
        "pe_warm": 8,          # PE pstate warmup matmuls during const DMA
        "fine_tail": 1,        # last tiles: split q/relu so phase 3 drains early
        "smx_sub": 12,
    }


def _build_program(cfg=None):
    if cfg is None:
        cfg = _default_cfg()
    p_odd_pool = int(cfg.get("p_odd_pool", 10))
    relu_act_diag = cfg.get("relu_act_diag", 19)      # int or per-tile list
    relu_pool_diag = cfg.get("relu_pool_diag", 0)     # int or per-tile list
    qmul_split = cfg.get("qmul_split", None)          # None or per-tile diag count on DVE
    smx_eng = cfg.get("smx_eng", "dve")              # sub engine: 'dve'|'pool'
    pe_warm = int(cfg.get("pe_warm", 0))              # warmup matmuls to ramp PE pstate
    fine_tail = int(cfg.get("fine_tail", 0))          # trailing tiles with split q/relu ops
    qadd_mode = cfg.get("qadd", "dve")
    qmul_eng = cfg.get("qmul", "dve")
    tile_order = cfg.get("tile_order", "bk")
    emission = cfg.get("emission", "tiles")
    qw_bufs = int(cfg.get("qw_bufs", 2))
    loop_reps = cfg.get("loop_reps", 0)
    wsplit = cfg.get("wsplit", 2)

    import concourse.bass as bass
    import concourse.mybir as mybir

    SplitDrainTileContext = _make_tc_class()

    dt = mybir.dt
    Alu = mybir.AluOpType
    Act = mybir.ActivationFunctionType

    nc = bass.Bass("TRN2", target_bir_lowering=False, debug=False)

    vp = nc.dram_tensor("vp", [128, ND * 128 * BL], dt.bfloat16, kind="ExternalInput")
    wa = nc.dram_tensor("wa", [ND, 128, H1E], dt.bfloat16, kind="ExternalInput")
    wc = nc.dram_tensor("wc", [ND, 128, H1E], dt.bfloat16, kind="ExternalInput")
    w2c = nc.dram_tensor("w2c", [H1 + 1, OUT], dt.bfloat16, kind="ExternalInput")
    w1cc = nc.dram_tensor("w1cc", [H1, 1], dt.float32, kind="ExternalInput")
    qr = nc.dram_tensor("qr", [BL, QC], dt.bfloat16, kind="ExternalInput")
    # [b, span-in-chunk, chunk*OUT+class]: keeps the store one large
    # contiguous-per-partition DMA per batch (128 x 4.8KB descriptors).
    out = nc.dram_tensor("out", [BL, L, NCH * OUT], dt.float32, kind="ExternalOutput")

    with SplitDrainTileContext(nc) as tc:
        import contextlib
        with contextlib.ExitStack() as ctx:
            const = ctx.enter_context(tc.tile_pool(name="const", bufs=1))
            combp = ctx.enter_context(tc.tile_pool(name="comb", bufs=1))
            hp = ctx.enter_context(tc.tile_pool(name="h", bufs=1))
            acp = ctx.enter_context(tc.tile_pool(name="acpsum", bufs=2, space="PSUM"))
            w2p = ctx.enter_context(tc.tile_pool(name="w2psum", bufs=1, space="PSUM"))
            smp = ctx.enter_context(tc.tile_pool(name="smx", bufs=1))
            qwp = ctx.enter_context(tc.tile_pool(name="qw", bufs=qw_bufs))

            # ---- constant loads: one wide DMA per logical tensor ------------
            vt_all = const.tile([128, ND * 128 * BL], dt.bfloat16, tag="vt")
            nc.sync.dma_start(out=vt_all[:], in_=vp.ap())
            wat_all = const.tile([128, ND * H1E], dt.bfloat16, tag="wat")
            wct_all = const.tile([128, ND * H1E], dt.bfloat16, tag="wct")
            # column-chunked so early k-tiles' matmuls can start before the
            # whole weight load lands (Tile tracks subtile deps); the small
            # tensors (qb/w2/w1) are issued right after the first chunk pair
            # so the q/assembly pipeline isn't gated on the bulk weight load
            if isinstance(wsplit, (list, tuple)):
                bounds = [0] + list(wsplit) + [H1E]
            else:
                bounds = [H1E * i // wsplit for i in range(wsplit + 1)]

            def _w_chunk(c0, c1):
                for t, src in ((wat_all, wa), (wct_all, wc)):
                    nc.sync.dma_start(
                        out=_view(t[:, :], c0, [(H1E, ND), (1, c1 - c0)]),
                        in_=_dram_view(src, c0, [(H1E, 128), (128 * H1E, ND), (1, c1 - c0)]),
                    )

            _w_chunk(bounds[0], bounds[1])
            # qb: both batches' q rows broadcast across HT partitions, one DMA
            qb_all = const.tile([HT, BL * QC], dt.bfloat16, tag="qb")
            nc.sync.dma_start(
                out=qb_all[:],
                in_=_dram_view(qr, 0, [(0, HT), (QC, BL), (1, QC)]),
            )
            # w2: block k = w2c[110k : 110k+kk]  ([111, 7*40] tile)
            w2_all = const.tile([HT + 1, NK * OUT], dt.bfloat16, tag="w2t")
            nc.sync.dma_start(
                out=w2_all[:],
                in_=_dram_view(w2c, 0, [(OUT, HT + 1), (HT * OUT, NK), (1, OUT)]),
            )
            # w1c scalars: [110, 7], col k = w1c[110k : 110k+110]
            w1_all = const.tile([HT, NK], dt.float32, tag="w1cs")
            nc.sync.dma_start(
                out=w1_all[:],
                in_=_dram_view(w1cc, 0, [(1, HT), (HT, NK)]),
            )
            for c0, c1 in zip(bounds[1:-1], bounds[2:]):
                _w_chunk(c0, c1)

            def vt(d):
                return _view(vt_all[:, :], 128 * BL * d, [(1, 128 * BL)])

            def wat(d, k):
                kk = HT + 1 if k == NK - 1 else HT
                return _view(wat_all[:, :], H1E * d + HT * k, [(1, kk)])

            def wct(d, k):
                kk = HT + 1 if k == NK - 1 else HT
                return _view(wct_all[:, :], H1E * d + HT * k, [(1, kk)])

            def w2t(k, kk):
                return _view(w2_all[0:kk, :], OUT * k, [(1, OUT)])

            if pe_warm:
                # dummy back-to-back matmuls during the const DMA window keep
                # the PE pstate ramp going so phase-1 matmuls run at full rate
                warm_sb = const.tile([128, 512], dt.bfloat16, tag="warm_sb")
                nc.gpsimd.memset(warm_sb[:], 0.0)
                warm_ps = acp.tile([HT + 1, 512], dt.float32, tag="acps")
                for _ in range(pe_warm):
                    nc.tensor.matmul(
                        warm_ps[:], lhsT=warm_sb[:, 0 : HT + 1], rhs=warm_sb[:],
                        start=True, stop=True,
                    )

            if loop_reps:
                ctx.enter_context(tc.For_i(0, loop_reps, 1))

            # ---- phase 1: AT/CT matmuls + bf16 copies -----------------------
            comb, sh = [], []
            for k in range(NK):
                kk = HT + 1 if k == NK - 1 else HT
                ps = acp.tile([kk, 512], dt.float32, tag="acps")
                for d in range(ND):
                    nc.tensor.matmul(
                        ps[:, 0 : 128 * BL],
                        lhsT=wat(d, k),
                        rhs=vt(d),
                        start=(d == 0),
                        stop=(d == ND - 1),
                    )
                for d in range(ND):
                    nc.tensor.matmul(
                        ps[:, 128 * BL : 256 * BL],
                        lhsT=wct(d, k),
                        rhs=vt(d),
                        start=(d == 0),
                        stop=(d == ND - 1),
                    )
                cb = combp.tile([kk, 2 * 128 + 2 * CEXT], dt.bfloat16, tag=f"comb{k}")
                # [A0 A1 | C0ext | C1ext]; Cext wraps the first 29 C columns
                # so rotated shifted reads never run off the end
                if cfg.get("cb_order", "c0_first") == "c0_first":
                    nc.scalar.copy(cb[:, 256:384], ps[:, 256:384])
                    nc.scalar.copy(cb[:, 384:413], ps[:, 256:285])
                    nc.scalar.copy(cb[:, 0:256], ps[:, 0:256])
                    nc.scalar.copy(cb[:, 256 + CEXT : 256 + CEXT + 128], ps[:, 384:512])
                    nc.scalar.copy(cb[:, 256 + CEXT + 128 : 256 + CEXT + 157], ps[:, 384:413])
                else:
                    nc.scalar.copy(cb[:, 0:384], ps[:, 0:384])
                    nc.scalar.copy(cb[:, 256 + CEXT : 256 + CEXT + 128], ps[:, 384:512])
                    nc.scalar.copy(cb[:, 384:413], ps[:, 256:285])
                    nc.scalar.copy(cb[:, 256 + CEXT + 128 : 256 + CEXT + 157], ps[:, 384:413])
                comb.append(cb)
                shk = []
                for b in range(BL):
                    s = combp.tile([kk, 156], dt.bfloat16, tag=f"sh{k}_{b}")
                    nc.vector.tensor_copy(
                        s[:], _view(cb[:, :], 256 + CEXT * b + 1, [(1, 156)])
                    )
                    shk.append(s)
                sh.append(shk)

            # ---- phase 2: assembly + q correction + relu --------------------
            ht = [[None] * NK for _ in range(BL)]
            if tile_order == "kb":
                tiles = [(b, k) for k in range(NK) for b in range(BL)]
            else:
                tiles = [(b, k) for b in range(BL) for k in range(NK)]
            for b, k in tiles:
                parts = HT + 1 if k == NK - 1 else HT
                h = hp.tile([parts, FDH], dt.bfloat16, tag=f"h{b}_{k}")
                ht[b][k] = h

            def emit_even(b, k):
                # even diagonals w = 0,2,..,28: CT[i + w] straight from comb
                kk = HT + 1 if k == NK - 1 else HT
                nc.vector.tensor_tensor(
                    out=_view(ht[b][k][0:kk, :], 0, [(256, NW2), (1, L)]),
                    in0=_view(comb[k][:, :], 128 * b, [(0, NW2), (1, L)]),
                    in1=_view(comb[k][:, :], 256 + CEXT * b, [(2, NW2), (1, L)]),
                    op=Alu.add,
                )

            def emit_odd(b, k):
                # odd diagonals w = 2m+1: CT[i + w] = sh[2m + i] (2x-aligned)
                kk = HT + 1 if k == NK - 1 else HT
                p = p_odd_pool
                if p > 0:
                    nc.gpsimd.tensor_tensor(
                        out=_view(ht[b][k][0:kk, :], 128, [(256, p), (1, L)]),
                        in0=_view(comb[k][:, :], 128 * b, [(0, p), (1, L)]),
                        in1=_view(sh[k][b][:, :], 0, [(2, p), (1, L)]),
                        op=Alu.add,
                    )
                if p < NW2:
                    nc.vector.tensor_tensor(
                        out=_view(ht[b][k][0:kk, :], 128 * (2 * p + 1), [(256, NW2 - p), (1, L)]),
                        in0=_view(comb[k][:, :], 128 * b, [(0, NW2 - p), (1, L)]),
                        in1=_view(sh[k][b][:, :], 2 * p, [(2, NW2 - p), (1, L)]),
                        op=Alu.add,
                    )

            def emit_q(b, k, ti):
                # qw = q * w1c[p] over the compact rotated window layout
                qw = qwp.tile([HT, QC], dt.bfloat16, tag="qw")
                nc.vector.tensor_scalar_mul(
                    qw[:], _view(qb_all[:, :], QC * b, [(1, QC)]),
                    w1_all[:, k : k + 1])
                return qw

            def emit_qadd(b, k, qw, w0=0, w1=29):
                # windows: diag w=wi+1 covers h slots [255+127*wi, +29)
                # (tail of diag w + harmless zero-bleed into diag w+1)
                if w0 == 0:
                    hv = ht[b][k][0:HT, 0:1]
                    nc.vector.tensor_tensor(
                        out=hv, in0=hv, in1=qw[:, 0:1], op=Alu.add)
                hv = _view(ht[b][k][0:HT, :], 255 + 127 * w0, [(127, w1 - w0), (1, 29)])
                qv = _view(qw[:, :], 1 + 29 * w0, [(29, w1 - w0), (1, 29)])
                nc.vector.tensor_tensor(out=hv, in0=hv, in1=qv, op=Alu.add)

            def emit_relu(b, k, ti, parts=None):
                kk = HT + 1 if k == NK - 1 else HT
                if parts is None:
                    rd = relu_act_diag[ti] if isinstance(relu_act_diag, (list, tuple)) else relu_act_diag
                    rp = relu_pool_diag[ti] if isinstance(relu_pool_diag, (list, tuple)) else relu_pool_diag
                    parts = [(0, rd, "dve"), (rd, rd + rp, "pool"), (rd + rp, WMAX, "act")]
                for d0, d1, eng in parts:
                    if d1 <= d0:
                        continue
                    hv = _view(ht[b][k][0:kk, :], L * d0, [(L, d1 - d0), (1, L)])
                    if eng == "dve":
                        nc.vector.tensor_scalar_max(hv, hv, 0.0)
                    elif eng == "pool":
                        nc.gpsimd.tensor_scalar_max(hv, hv, 0.0)
                    else:
                        nc.scalar.activation(hv, hv, Act.Relu)

            ntiles = len(tiles)
            if cfg.get("skip_phase2"):
                tiles = []
            for ti, (b, k) in enumerate(tiles):
                emit_even(b, k)
                emit_odd(b, k)
                if ti >= ntiles - fine_tail:
                    # trailing tiles: split q/relu by diagonal halves so
                    # phase-3 chunk matmuls unlock progressively
                    MID = 15
                    qw = emit_q(b, k, ti)
                    emit_qadd(b, k, qw, 0, MID)
                    emit_relu(b, k, ti, parts=[(0, MID + 1, "dve")])
                    emit_qadd(b, k, qw, MID, 29)
                    emit_relu(b, k, ti, parts=[(MID + 1, WMAX, "dve")])
                else:
                    qw = emit_q(b, k, ti)
                    emit_qadd(b, k, qw)
                    emit_relu(b, k, ti)

            # ---- phase 3: W2 matmul + log_softmax + store -------------------
            if cfg.get("skip_phase3"):
                groups = []
            else:
                groups = cfg.get("groups", [(0, 12), (12, 12), (24, NCH - 24)])
            sg = int(cfg.get("smx_sub", 12))   # softmax/store every sg chunks
            for b in range(BL):
                fin = smp.tile([128, NCH * OUT], dt.float32, tag=f"fin{b}")
                ex = smp.tile([128, NCH * OUT], dt.float32, tag=f"ex{b}")
                ss = smp.tile([128, NCH], dt.float32, tag=f"ss{b}")
                lse = smp.tile([128, NCH], dt.float32, tag=f"lse{b}")
                pts = []
                for g in range(len(groups)):
                    pt = w2p.tile([128, 480], dt.float32, tag=f"w2ps_{b}_{g}")
                    pts.append(pt)
                # chunk-outer: each chunk's 7-matmul accumulation group runs
                # contiguously (interleaving start groups within a PSUM bank
                # corrupts accumulation), softmax per group as soon as its
                # chunks complete.
                for g, (c0, n) in enumerate(groups):
                    for j in range(n):
                        c = c0 + j
                        for k in range(NK):
                            kk = HT + 1 if k == NK - 1 else HT
                            nc.tensor.matmul(
                                pts[g][:, OUT * j : OUT * j + OUT],
                                lhsT=ht[b][k][0:kk, L * c : L * c + L],
                                rhs=w2t(k, kk),
                                start=(k == 0),
                                stop=(k == NK - 1),
                            )
                for g, (g0, gn) in enumerate(groups):
                    for c0 in range(g0, g0 + gn, sg):
                        n = min(sg, g0 + gn - c0)
                        po = OUT * (c0 - g0)
                        nc.scalar.activation(
                            ex[:, OUT * c0 : OUT * (c0 + n)],
                            pts[g][:, po : po + OUT * n],
                            Act.Exp,
                        )
                        nc.vector.tensor_reduce(
                            out=ss[:, c0 : c0 + n],
                            in_=_view(ex[:, :], OUT * c0, [(OUT, n), (1, OUT)]),
                            axis=mybir.AxisListType.X,
                            op=Alu.add,
                        )
                        nc.scalar.activation(
                            lse[:, c0 : c0 + n], ss[:, c0 : c0 + n], Act.Ln
                        )
                        nc.vector.tensor_tensor(
                            out=_view(fin[:, :], OUT * c0, [(1, OUT), (OUT, n)]),
                            in0=_view(pts[g][:, :], po, [(1, OUT), (OUT, n)]),
                            in1=_view(lse[:, :], c0, [(0, OUT), (1, n)]),
                            op=Alu.subtract,
                        )
                        nc.sync.dma_start(
                            out=out[b][:, OUT * c0 : OUT * (c0 + n)],
                            in_=fin[:, OUT * c0 : OUT * (c0 + n)],
                        )

    return nc


def _host_prep(hidden_states, pred_spans, token_num, mask, W1, b1, W2, b2):
    hs = _f32(hidden_states)
    pred = np.asarray(pred_spans)
    W1 = _f32(W1)
    b1 = _f32(b1)
    W2f = _f32(W2)
    b2 = _f32(b2)
    tn = int(token_num)

    vecs = hs[:, 1 : tn + 1, :]                     # [B, L, D]
    W1a, W1b, w1c = W1[:D], W1[D : 2 * D], W1[2 * D]

    # per-core packed, augmented, transposed activations.  Column H1 (=770)
    # of both weight matrices hits only the all-ones augmented row with
    # weight 0.5 each, so AT[770,:] + CT[770,:] == 1.0 -- the b2 ones-row of
    # h materializes through the normal assembly pipeline.
    in_maps = []
    w1a_aug = np.zeros((DAUG, H1E), np.float32)
    w1a_aug[0:D, 0:H1] = W1a
    w1a_aug[D, 0:H1] = w1c
    w1a_aug[D + 1, 0:H1] = b1
    w1a_aug[D + 1, H1] = 0.5
    w1c_aug = np.zeros((DAUG, H1E), np.float32)
    w1c_aug[0:D, 0:H1] = W1b
    w1c_aug[D + 2, 0:H1] = -w1c
    w1c_aug[D + 1, H1] = 0.5
    wa_np = _bf16(w1a_aug.reshape(ND, 128, H1E))
    wc_np = _bf16(w1c_aug.reshape(ND, 128, H1E))
    w2cat = np.zeros((H1 + 1, OUT), np.float32)
    w2cat[0:H1] = W2f
    w2cat[H1] = b2
    w2_np = _bf16(w2cat)
    w1cc_np = _f32(w1c.reshape(H1, 1))

    ii = np.arange(L)
    q_region_ok = True
    for c in range(NCORES):
        va = np.zeros((DAUG, 128 * BL), np.float32)
        qc = np.zeros((BL, QC), np.float32)
        for b in range(BL):
            gb = BL * c + b
            s, e = int(pred[gb, 0]), int(pred[gb, 1])
            # per-batch token rotation by e: device column i' = token (i'+e)%L.
            # This pins the q-correction support to static windows.
            rot = (ii + e) % L
            va[0:D, 128 * b : 128 * b + L] = vecs[gb].T[:, rot]
            va[D, 128 * b : 128 * b + L] = (rot >= s).astype(np.float32)
            va[D + 1, 128 * b : 128 * b + L] = 1.0
            va[D + 2, 128 * b : 128 * b + L] = (rot > e).astype(np.float32)
            qrow = np.zeros(FDH, np.float32)
            for w in range(WMAX):
                i = ii[: L - w]
                q = ((i < s) & (i + w > e)).astype(np.float32)
                if e - s == w and s < L - w:
                    q[s] += 1.0
                qrow[w * L + (i - e) % L] = q
            # compact: col0 = (diag0, i'=0); then 29 windows of 29
            qc[b, 0] = qrow[0]
            win = qrow[255 : 255 + 127 * 28 + 29].copy()
            qc[b, 1:] = np.lib.stride_tricks.as_strided(
                win, (29, 29), (127 * 4, 4)).reshape(-1)
            # safety: everything outside the windows must be zero
            chk = qrow.copy()
            chk[0] = 0
            for wi in range(29):
                chk[255 + 127 * wi : 255 + 127 * wi + 29] = 0
            if chk.any():
                q_region_ok = False
        in_maps.append(
            dict(
                vp=_bf16(np.ascontiguousarray(
                    va.reshape(ND, 128, 128 * BL).transpose(1, 0, 2)
                ).reshape(128, ND * 128 * BL)),
                wa=wa_np,
                wc=wc_np,
                w2c=w2_np,
                w1cc=w1cc_np,
                qr=_bf16(qc),
            )
        )
    return in_maps if q_region_ok else None


def _fast_path_ok(hidden_states, pred_spans, token_num, mask):
    hs = np.asarray(hidden_states)
    mask = np.asarray(mask)
    if hs.shape != (B, L + 1, D) or int(token_num) != L:
        return False
    if mask.shape != (L, L):
        return False
    vi, vj = np.nonzero(mask == 1)
    w = vj - vi
    if len(vi) == 0 or w.min() < 0 or w.max() != WMAX - 1:
        return False
    want = sum(L - ww for ww in range(WMAX))
    if len(vi) != want:
        return False
    for ww in range(WMAX):
        sel = vi[w == ww]
        if len(sel) != L - ww or not np.array_equal(np.sort(sel), np.arange(L - ww)):
            return False
    return True


def _reference_numpy(hidden_states, pred_spans, token_num, mask, W1, b1, W2, b2):
    """Exact fallback (host only) for input shapes the device program
    doesn't cover; mirrors reference.py semantics."""
    hs = _f32(hidden_states)
    mask = np.asarray(mask)
    tn = int(token_num)
    vi, vj = np.nonzero(mask == 1)
    vecs = hs[:, 1 : tn + 1, :]
    n = vecs.shape[1]
    vic = np.clip(vi, 0, n - 1)
    vjc = np.clip(vj, 0, n - 1)
    xi = vecs[:, vic, :]
    xj = vecs[:, vjc, :]
    s = np.asarray(pred_spans)[:, 0:1]
    e = np.asarray(pred_spans)[:, 1:2]
    exact = (vi[None, :] == s) & (vj[None, :] == e)
    inside = (vi[None, :] >= s) & (vj[None, :] <= e) & (vi[None, :] <= vj[None, :])
    ind = np.where(exact, 2.0, np.where(inside, 1.0, 0.0)).astype(np.float32)
    W1 = _f32(W1)
    Dd = vecs.shape[2]
    h = xi @ W1[:Dd] + xj @ W1[Dd : 2 * Dd] + ind[..., None] * W1[2 * Dd] + _f32(b1)
    h = np.maximum(h, 0.0)
    logits = h @ _f32(W2) + _f32(b2)
    m = logits.max(axis=-1, keepdims=True)
    z = np.exp(logits - m)
    return (logits - m - np.log(z.sum(axis=-1, keepdims=True))).astype(np.float32)


def kernel(**inputs):
    hidden_states = inputs["hidden_states"]
    pred_spans = inputs["pred_spans"]
    token_num = inputs["token_num"]
    mask = inputs["span_available_indication_matrix"]
    W1, b1, W2, b2 = inputs["W1"], inputs["b1"], inputs["W2"], inputs["b2"]

    if not _fast_path_ok(hidden_states, pred_spans, token_num, mask):
        return _reference_numpy(
            hidden_states, pred_spans, token_num, mask, W1, b1, W2, b2
        )

    from concourse.bass_utils import run_bass_kernel_spmd

    key = "v4"  # static program: depends only on shapes, never on values
    if key not in _prog_cache:
        _prog_cache[key] = _build_program(_default_cfg())
    nc = _prog_cache[key]

    in_maps = _host_prep(
        hidden_states, pred_spans, token_num, mask, W1, b1, W2, b2
    )
    if in_maps is None:
        return _reference_numpy(
            hidden_states, pred_spans, token_num, mask, W1, b1, W2, b2
        )
    res = run_bass_kernel_spmd(nc, in_maps, list(range(NCORES)))
    kernel.last_results = res

    # gather + un-permute: device emits [BL, span-in-chunk(=i'), chunk(=w), OUT]
    # where i' = (i - e_b) % L (per-batch rotated token coordinate)
    mask = np.asarray(mask)
    pred = np.asarray(pred_spans)
    vi, vj = np.nonzero(mask == 1)
    out = np.empty((B, len(vi), OUT), np.float32)
    for c in range(NCORES):
        o = (
            res.results[c]["out"]
            .reshape(BL, L, NCH, OUT)
            .transpose(0, 2, 1, 3)
            .reshape(BL, FDH, OUT)
        )
        for b in range(BL):
            e = int(pred[BL * c + b, 1])
            perm = (vj - vi) * L + (vi - e) % L
            out[BL * c + b] = o[b][perm]
    return out

